# revision 23
# baseline (speedup 1.0000x reference)
"""Trainium2 Bass kernel for nn_MACAM (cross-attn modulation + instance norm).

Pure data parallel: batch B=16, 2 samples per core over 8 NeuronCores.
bf16 everywhere except the argmax/scores block (kept fp32 for exact argmax).

  - h shipped bf16 (halves input DMA); output written bf16, host converts back
    to fp32; weights folded+cast on host: M1 = fc_k_w.T @ conv_w so
    kc = ws@M1 + v1, attn = kcT-chunks @ h + kb, kb = ws@u + w0.
  - instance norm folded into the modulation weights:
        gamma2[l,c] = gamma[l,c]*alpha[c]
        beta2[l,c]  = beta[l,c] + gamma[l,c]*delta[c]
        out = h (.) (gamma2.T@attn) + beta2.T@attn
  - argmax/gather as a one-hot matmul (is_equal against the row max).
  - PSUM lives in two 4-bank FIFO pools ([128,1024] f32 slots) shared by all
    phases; modulation runs 1024-wide pairs with a LAG-2 software pipeline so
    the PE never waits on the DVE multiply.
  - PSUM egress on ACT; the final add is split across Pool/PE(id-matmul)/DVE
    to balance engines; sample-1 front work (scores/attn/bn_stats) is emitted
    into hook slots inside sample-0's modulation loop to keep PE/DVE packed.
  - NOTE: sustained 8-core load trips the activity throttler (PE clamped to
    ~1.2 GHz after ~10us); per-engine budgets assume the clamped clock.
    fp8 maps were tested offline and fail the 2e-2 gate (~4e-2); bf16 path
    measures ~6.7e-3.
"""

import os
import sys

os.environ.setdefault("MYCRO_LOCAL_CACHE", "1")
sys.path.insert(0, "/opt/trn_rl_repo")

import ml_dtypes
import numpy as np

import concourse.bacc as bacc
import concourse.bass as bass
import concourse.mybir as mybir
import concourse.tile as tile
from concourse.bass_utils import run_bass_kernel_spmd

N_CORES = 8
B, C, H, W = 16, 512, 64, 64
HW = H * W
L, D, Q = 64, 512, 512
S = B // N_CORES          # samples per core
EPS = 1e-5
NP = 8                    # HW pieces of 512
NC4 = 4                   # channel chunks of 128

f32 = mybir.dt.float32
bf16 = mybir.dt.bfloat16
AF = mybir.ActivationFunctionType
ALU = mybir.AluOpType
AX = mybir.AxisListType
BF = ml_dtypes.bfloat16


def _build_program():
    nc = bacc.Bacc("TRN2", target_bir_lowering=False, debug=False,
                   num_devices=N_CORES)
    dt_ = nc.dram_tensor
    h_d = dt_("h_bf", [S, C, H, W], bf16, kind="ExternalInput").ap()
    wsT_d = dt_("ws_t", [S, D, L], f32, kind="ExternalInput").ap()
    wtT_d = dt_("wt_t", [S, D, L], f32, kind="ExternalInput").ap()
    wsTb_d = dt_("ws_t_bf", [S, D, L], bf16, kind="ExternalInput").ap()
    wt_d = dt_("wt", [S, L, D], f32, kind="ExternalInput").ap()
    m1_d = dt_("m1_bf", [D, C], bf16, kind="ExternalInput").ap()
    v14_d = dt_("v1_4", [128, 4], f32, kind="ExternalInput").ap()
    u4_d = dt_("u4", [128, 4], f32, kind="ExternalInput").ap()
    w0_d = dt_("w0_col", [128, 1], f32, kind="ExternalInput").ap()
    fw_d = dt_("fw_bf", [D, 2 * C], bf16, kind="ExternalInput").ap()
    fcb_d = dt_("fcb_row_bf", [1, 2 * C], bf16, kind="ExternalInput").ap()
    ones1_d = dt_("ones1_bf", [1, L], bf16, kind="ExternalInput").ap()
    inw_d = dt_("inw_col", [128, 4], f32, kind="ExternalInput").ap()
    inb_d = dt_("inb_col", [128, 4], f32, kind="ExternalInput").ap()
    epz_d = dt_("eps_zero", [128, 2], f32, kind="ExternalInput").ap()
    id64_d = dt_("identity64", [L, L], f32, kind="ExternalInput").ap()
    nid64_d = dt_("negid64", [L, L], f32, kind="ExternalInput").ap()
    ones64_d = dt_("ones64", [L, L], f32, kind="ExternalInput").ap()
    id128f_d = dt_("identity128f", [128, 128], f32, kind="ExternalInput").ap()
    id128b_d = dt_("identity128b", [128, 128], bf16, kind="ExternalInput").ap()
    sel8_d = dt_("sel8_bf", [8, 8 * L], bf16, kind="ExternalInput").ap()
    out_d = dt_("out", [S, C, H, W], bf16, kind="ExternalOutput").ap()

    h_v = h_d.rearrange("s (n p) a b -> s n p (a b)", p=128)     # [S,4,128,4096]
    out_v = out_d.rearrange("s (n p) a b -> s n p (a b)", p=128)
    wsT_v = wsT_d.rearrange("s (n p) l -> s p n l", p=128)       # [S,128,4,64]
    wtT_v = wtT_d.rearrange("s (n p) l -> s p n l", p=128)
    wsTb_v = wsTb_d.rearrange("s (n p) l -> s p n l", p=128)

    with tile.TileContext(nc) as tc:
        with (
            tc.tile_pool(name="wpool", bufs=1) as wpool,
            tc.tile_pool(name="hpool", bufs=8) as hpool,
            tc.tile_pool(name="attnpool", bufs=2) as attnpool,
            tc.tile_pool(name="spool", bufs=2) as spool,
            tc.tile_pool(name="piece", bufs=3) as piece,
            tc.tile_pool(name="stage", bufs=2) as stage,
            tc.tile_pool(name="ps_g", bufs=2, space="PSUM") as ps_g,
            tc.tile_pool(name="ps_b", bufs=2, space="PSUM") as ps_b,
        ):
            def gslot():
                t = ps_g.tile([128, 1024], f32, tag="g", name="gs")
                return t

            def bslot():
                t = ps_b.tile([128, 1024], f32, tag="b", name="bs")
                return t

            # ---- persistent weights ----
            m1_t = []     # M1 chunks [128(d), 512(c)] bf16
            fw_t = []     # fc_w.T chunks [128(d), 1024] bf16
            for j in range(4):
                t = wpool.tile([128, C], bf16, tag=f"m1{j}", name=f"m1{j}")
                nc.scalar.dma_start(t[:], m1_d[j * 128:(j + 1) * 128, :])
                m1_t.append(t)
            for j in range(4):
                t = wpool.tile([128, 2 * C], bf16, tag=f"fw{j}", name=f"fw{j}")
                nc.scalar.dma_start(t[:], fw_d[j * 128:(j + 1) * 128, :])
                fw_t.append(t)
            v1_col = wpool.tile([128, 4], f32, tag="v1")
            nc.gpsimd.dma_start(v1_col[:], v14_d)
            u_col = wpool.tile([128, 4], f32, tag="u")
            nc.gpsimd.dma_start(u_col[:], u4_d)
            w0_col = wpool.tile([128, 1], f32, tag="w0")
            nc.gpsimd.dma_start(w0_col[:], w0_d)
            fcb_row = wpool.tile([1, 2 * C], bf16, tag="fcb")
            nc.gpsimd.dma_start(fcb_row[:], fcb_d)
            ones1b = wpool.tile([1, L], bf16, tag="ones1b")
            nc.gpsimd.dma_start(ones1b[:], ones1_d)
            inw_col = wpool.tile([128, 4], f32, tag="inw")
            nc.gpsimd.dma_start(inw_col[:], inw_d)
            inb_col = wpool.tile([128, 4], f32, tag="inb")
            nc.gpsimd.dma_start(inb_col[:], inb_d)
            epz = wpool.tile([128, 2], f32, tag="epz")
            nc.gpsimd.dma_start(epz[:], epz_d)
            id64 = wpool.tile([L, L], f32, tag="id64")
            nc.gpsimd.dma_start(id64[:], id64_d)
            nid64 = wpool.tile([L, L], f32, tag="nid64")
            nc.gpsimd.dma_start(nid64[:], nid64_d)
            ones64 = wpool.tile([L, L], f32, tag="ones64")
            nc.gpsimd.dma_start(ones64[:], ones64_d)
            id128f = wpool.tile([128, 128], f32, tag="id128f")
            nc.gpsimd.dma_start(id128f[:], id128f_d)
            id128b = wpool.tile([128, 128], bf16, tag="id128b")
            nc.gpsimd.dma_start(id128b[:], id128b_d)
            sel8 = wpool.tile([8, 8 * L], bf16, tag="sel8")
            nc.gpsimd.dma_start(sel8[:], sel8_d)

            def load_h(s):
                h_t = []
                for cc in range(NC4):
                    t = hpool.tile([128, HW], bf16, tag="h", name="h")
                    nc.sync.dma_start(t[:], h_v[s, cc])
                    h_t.append(t)
                return h_t

            def fA(s, h_t=None):
                st = {}
                st["h_t"] = h_t if h_t is not None else load_h(s)

                # ---- per-sample small inputs ----
                wsT4 = spool.tile([128, 4 * L], f32, tag="wsT4")
                nc.gpsimd.dma_start(
                    wsT4[:].rearrange("p (n l) -> p n l", l=L), wsT_v[s])
                wtT4 = spool.tile([128, 4 * L], f32, tag="wtT4")
                nc.gpsimd.dma_start(
                    wtT4[:].rearrange("p (n l) -> p n l", l=L), wtT_v[s])
                wsT4b = spool.tile([128, 4 * L], bf16, tag="wsT4b")
                nc.gpsimd.dma_start(
                    wsT4b[:].rearrange("p (n l) -> p n l", l=L), wsTb_v[s])
                wt_sb = spool.tile([L, D], f32, tag="wt_sb")
                nc.gpsimd.dma_start(wt_sb[:], wt_d[s])
                st["wsT4"] = wsT4
                st["st_col"] = spool.tile([128, 8], f32, tag="st_col",
                                          name="st_col")

                # ---- masking allocation (fp32): scores -> one-hot P ----
                scores_ps = gslot()[0:L, 0:L]
                for j in range(4):
                    nc.tensor.matmul(
                        scores_ps,
                        wsT4[:, j * L:(j + 1) * L], wtT4[:, j * L:(j + 1) * L],
                        start=(j == 0), stop=(j == 3))
                scores_sb = spool.tile([L, L], f32, tag="scores_sb")
                nc.scalar.copy(scores_sb[:], scores_ps)
                colsum_ps = bslot()[0:1, 0:L]
                nc.tensor.matmul(colsum_ps, ones64[:, 0:1], scores_sb[:],
                                 start=True, stop=True)
                colsum_row = spool.tile([1, L], f32, tag="colsum")
                nc.scalar.copy(colsum_row[:], colsum_ps)
                left_ps = gslot()[0:L, 0:L]
                nc.tensor.matmul(left_ps, ones64[0:1, :], colsum_row[:],
                                 start=True, stop=False)
                nc.tensor.matmul(left_ps, nid64[:], scores_sb[:],
                                 start=False, stop=True)
                rowmax = spool.tile([L, 1], f32, tag="rowmax")
                nc.vector.tensor_reduce(rowmax[:], left_ps, AX.X, ALU.max)
                P_sb = spool.tile([L, L], f32, tag="P_sb")
                nc.vector.tensor_scalar(P_sb[:], left_ps, rowmax[:], None,
                                        ALU.is_equal)
                PT_ps = bslot()[0:L, 0:L]
                nc.tensor.transpose(PT_ps, P_sb[:], id64[:])
                PT_sb = spool.tile([L, L], f32, tag="PT_sb")
                nc.scalar.copy(PT_sb[:], PT_ps)

                # ---- w_allocT = wt.T @ P.T, cast to bf16 ----
                waT_pack = bslot()
                waT_bf = spool.tile([128, 4 * L], bf16, tag="waT_bf")
                for j in range(4):
                    nc.tensor.matmul(waT_pack[:, j * L:(j + 1) * L],
                                     wt_sb[:, j * 128:(j + 1) * 128],
                                     PT_sb[:], start=True, stop=True)
                nc.scalar.copy(waT_bf[:], waT_pack[:, 0:4 * L])

                # ---- bg = w_alloc @ fc_w.T + fc_b ----
                beta_sb = spool.tile([L, C], f32, tag="beta_sb")
                gamma_sb = spool.tile([L, C], f32, tag="gamma_sb")
                for half, dst in ((0, beta_sb), (1, gamma_sb)):
                    bg_ps = bslot()[0:L, 0:C]
                    for j in range(4):
                        nc.tensor.matmul(
                            bg_ps, waT_bf[:, j * L:(j + 1) * L],
                            fw_t[j][:, half * C:(half + 1) * C],
                            start=(j == 0), stop=False)
                    nc.tensor.matmul(bg_ps, ones1b[:],
                                     fcb_row[:, half * C:(half + 1) * C],
                                     start=False, stop=True)
                    nc.scalar.copy(dst[:], bg_ps)
                st["beta_sb"] = beta_sb
                st["gamma_sb"] = gamma_sb

                # ---- kcT = M1.T @ ws.T + v1 ----
                kcT_pack = bslot()
                kcT_bf = spool.tile([128, 4 * L], bf16, tag="kcT_bf")
                for cc in range(NC4):
                    for j in range(4):
                        nc.tensor.matmul(
                            kcT_pack[:, cc * L:(cc + 1) * L],
                            m1_t[j][:, cc * 128:(cc + 1) * 128],
                            wsT4b[:, j * L:(j + 1) * L],
                            start=(j == 0), stop=(j == 3))
                    nc.scalar.activation(kcT_bf[:, cc * L:(cc + 1) * L],
                                         kcT_pack[:, cc * L:(cc + 1) * L],
                                         AF.Identity,
                                         bias=v1_col[:, cc:cc + 1])
                st["kcT_bf"] = kcT_bf

                # ---- kb = ws @ u + w0 ----
                kb_ps = bslot()[0:L, 0:1]
                for j in range(4):
                    nc.tensor.matmul(kb_ps, wsT4[:, j * L:(j + 1) * L],
                                     u_col[:, j:j + 1],
                                     start=(j == 0), stop=(j == 3))
                kb_col = spool.tile([L, 1], f32, tag="kb_col")
                nc.scalar.activation(kb_col[:], kb_ps, AF.Identity,
                                     bias=w0_col[0:L, :])
                st["kb_col"] = kb_col
                st["attn_bf"] = attnpool.tile([L, HW], bf16, tag="attn_bf",
                                              name="attn_bf")
                return st

            def attn_piece(st, p2):
                attn2 = gslot()
                for half in range(2):
                    pp = 2 * p2 + half
                    for cc in range(NC4):
                        nc.tensor.matmul(
                            attn2[0:L, half * 512:(half + 1) * 512],
                            st["kcT_bf"][:, cc * L:(cc + 1) * L],
                            st["h_t"][cc][:, pp * 512:(pp + 1) * 512],
                            start=(cc == 0), stop=(cc == 3))
                nc.scalar.activation(
                    st["attn_bf"][:, p2 * 1024:(p2 + 1) * 1024],
                    attn2[0:L, :], AF.Identity, bias=st["kb_col"][:])

            def stats_chunk_act(st, cc):
                # ACT-side stats: sum & sumsq via activation accum_out.
                # Used for sample-1 chunks during maps0, where ACT has slack
                # and the DVE is the pacer.
                st_col = st["st_col"]
                h = st["h_t"][cc]
                scr = spool.tile([128, HW], bf16, tag="scr", name="scr")
                sq_col = spool.tile([128, 1], f32, tag="sq_col", name="sq")
                nc.scalar.activation(scr[:], h[:], AF.Square,
                                     accum_out=sq_col[:])
                sum_col = spool.tile([128, 1], f32, tag="sum_col", name="sum")
                nc.scalar.activation(scr[:], h[:], AF.Identity,
                                     accum_out=sum_col[:])
                mean = spool.tile([128, 1], f32, tag="mean", name="mean")
                nc.vector.tensor_scalar(mean[:], sum_col[:], 1.0 / HW, None,
                                        ALU.mult)
                ms2 = spool.tile([128, 1], f32, tag="ms2", name="ms2")
                nc.vector.tensor_tensor(ms2[:], mean[:], mean[:], ALU.mult)
                var = spool.tile([128, 1], f32, tag="var", name="var")
                nc.vector.tensor_scalar(var[:], sq_col[:], 1.0 / HW, None,
                                        ALU.mult)
                nc.vector.tensor_tensor(var[:], var[:], ms2[:], ALU.subtract)
                sd = spool.tile([128, 1], f32, tag="sd", name="sd")
                nc.scalar.activation(sd[:], var[:], AF.Sqrt, bias=epz[:, 0:1])
                rs = spool.tile([128, 1], f32, tag="rs", name="rs")
                nc.vector.reciprocal(rs[:], sd[:])
                nc.vector.tensor_tensor(
                    st_col[:, cc:cc + 1], rs[:], inw_col[:, cc:cc + 1],
                    ALU.mult)
                ms = spool.tile([128, 1], f32, tag="ms", name="ms")
                nc.vector.tensor_tensor(ms[:], mean[:],
                                        st_col[:, cc:cc + 1], ALU.mult)
                nc.vector.tensor_tensor(st_col[:, 4 + cc:5 + cc],
                                        inb_col[:, cc:cc + 1], ms[:],
                                        ALU.subtract)

            def stats_chunk(st, cc):
                st_col = st["st_col"]
                st6 = spool.tile([128, 48], f32, tag="st6", name="st6")
                for k in range(8):
                    nc.vector.bn_stats(
                        st6[:, k * 6:(k + 1) * 6],
                        st["h_t"][cc][:, k * 512:(k + 1) * 512])
                mv = spool.tile([128, 2], f32, tag="mv", name="mv")
                nc.vector.bn_aggr(mv[:], st6[:])
                sd = spool.tile([128, 1], f32, tag="sd", name="sd")
                nc.scalar.activation(sd[:], mv[:, 1:2], AF.Sqrt,
                                     bias=epz[:, 0:1])
                rs = spool.tile([128, 1], f32, tag="rs", name="rs")
                nc.vector.reciprocal(rs[:], sd[:])
                nc.vector.tensor_tensor(
                    st_col[:, cc:cc + 1], rs[:], inw_col[:, cc:cc + 1],
                    ALU.mult)
                ms = spool.tile([128, 1], f32, tag="ms", name="ms")
                nc.vector.tensor_tensor(ms[:], mv[:, 0:1],
                                        st_col[:, cc:cc + 1], ALU.mult)
                nc.vector.tensor_tensor(st_col[:, 4 + cc:5 + cc],
                                        inb_col[:, cc:cc + 1], ms[:],
                                        ALU.subtract)

            def folds(st):
                stT_ps = bslot()[0:8, 0:128]
                nc.tensor.transpose(stT_ps, st["st_col"][:], id128f[:])
                st8b = spool.tile([8, 128], bf16, tag="st8b")
                nc.scalar.copy(st8b[:], stT_ps)
                sm_ps = gslot()[0:L, 0:C]
                t2m_ps = bslot()[0:L, 0:C]
                for j in range(4):
                    nc.tensor.matmul(sm_ps[:, j * 128:(j + 1) * 128],
                                     sel8[:, j * L:(j + 1) * L], st8b[:],
                                     start=True, stop=True)
                    nc.tensor.matmul(t2m_ps[:, j * 128:(j + 1) * 128],
                                     sel8[:, (4 + j) * L:(5 + j) * L], st8b[:],
                                     start=True, stop=True)
                gamma2_bf = spool.tile([L, C], bf16, tag="gamma2_bf",
                                       name="gamma2_bf")
                nc.vector.tensor_tensor(gamma2_bf[:], st["gamma_sb"][:],
                                        sm_ps[:], ALU.mult)
                tg_sb = spool.tile([L, C], f32, tag="tg_sb")
                nc.vector.tensor_tensor(tg_sb[:], st["gamma_sb"][:],
                                        t2m_ps[:], ALU.mult)
                beta2_bf = spool.tile([L, C], bf16, tag="beta2_bf",
                                      name="beta2_bf")
                nc.vector.tensor_tensor(beta2_bf[:], st["beta_sb"][:],
                                        tg_sb[:], ALU.add)
                st["gamma2_bf"] = gamma2_bf
                st["beta2_bf"] = beta2_bf

            def maps(s, st, hooks=None):
                h_t = st["h_t"]
                attn_bf = st["attn_bf"]
                gamma2_bf = st["gamma2_bf"]
                beta2_bf = st["beta2_bf"]
                # ---- modulation: 1024-wide pairs, LAG-deep software pipeline
                # (PE never waits on the DVE mult; PSUM slots recycle 2 pairs
                # = 4 pieces apart) ----
                LAG = 2
                pairs = [(cc, p2) for cc in range(NC4) for p2 in range(4)]
                stage_t = {}
                tmp_tiles = {}
                for t in range(len(pairs) + LAG):
                    if hooks and t in hooks:
                        for fn in hooks.pop(t):
                            fn()
                    if t < len(pairs):
                        cc, p2 = pairs[t]
                        if p2 == 0:
                            stage_t[cc] = stage.tile([128, HW], bf16,
                                                     tag="stage", name="stg")
                        gm2 = gslot()
                        for half in range(2):
                            pp = 2 * p2 + half
                            nc.tensor.matmul(
                                gm2[:, half * 512:(half + 1) * 512],
                                gamma2_bf[:, cc * 128:(cc + 1) * 128],
                                attn_bf[:, pp * 512:(pp + 1) * 512],
                                start=True, stop=True)
                        tmp2 = piece.tile([128, 1024], bf16, tag="tmp")
                        nc.vector.tensor_tensor(
                            tmp2[:], h_t[cc][:, p2 * 1024:(p2 + 1) * 1024],
                            gm2[:], ALU.mult)
                        tmp_tiles[t] = tmp2
                    if t >= LAG:
                        i = t - LAG
                        cc, p2 = pairs[i]
                        # adds: mostly Pool; 2/sample on PE (id-matmul),
                        # 2/sample on DVE (bf16 2x) to balance engines
                        if hooks is not None:
                            # stats of the next sample share the DVE here
                            eng = "pe" if i % 4 == 3 else "pool"
                        else:
                            eng = "pe" if i % 8 == 3 else (
                                "dve" if (i % 4 == 3 or i >= 13) else "pool")
                        t2 = tmp_tiles.pop(i)
                        sl = stage_t[cc][:, p2 * 1024:(p2 + 1) * 1024]
                        bm2 = bslot()
                        for half in range(2):
                            pp = 2 * p2 + half
                            nc.tensor.matmul(
                                bm2[:, half * 512:(half + 1) * 512],
                                beta2_bf[:, cc * 128:(cc + 1) * 128],
                                attn_bf[:, pp * 512:(pp + 1) * 512],
                                start=True, stop=(eng != "pe"))
                        if eng == "pe":
                            for half in range(2):
                                nc.tensor.matmul(
                                    bm2[:, half * 512:(half + 1) * 512],
                                    id128b[:],
                                    t2[:, half * 512:(half + 1) * 512],
                                    start=False, stop=True)
                            nc.scalar.copy(sl, bm2[:])
                        else:
                            bmb = piece.tile([128, 1024], bf16, tag="bmb")
                            nc.scalar.copy(bmb[:], bm2[:])
                            e = nc.gpsimd if eng == "pool" else nc.vector
                            e.tensor_tensor(sl, bmb[:], t2[:], ALU.add)
                        nc.sync.dma_start(
                            out_v[s, cc][:, p2 * 1024:(p2 + 1) * 1024],
                            stage_t[cc][:, p2 * 1024:(p2 + 1) * 1024])

            st0 = fA(0)
            h1 = load_h(1)
            for p2 in range(4):
                attn_piece(st0, p2)
            stats_chunk(st0, 0)
            for cc in range(1, NC4):
                stats_chunk(st0, cc)
            folds(st0)

            st1 = {}

            def _fa1():
                st1.update(fA(1, h1))

            hooks = {
                1: [_fa1],
                2: [lambda: stats_chunk(st1, 0)],
                4: [lambda: attn_piece(st1, 0)],
                5: [lambda: stats_chunk(st1, 1)],
                7: [lambda: attn_piece(st1, 1),
                    lambda: stats_chunk_act(st1, 2)],
                10: [lambda: attn_piece(st1, 2)],
                11: [lambda: stats_chunk_act(st1, 3)],
                13: [lambda: attn_piece(st1, 3)],
                15: [lambda: folds(st1)],
            }
            maps(0, st0, hooks)
            maps(1, st1)

    nc.compile()
    return nc


_NC_CACHE = None


def _get_nc():
    global _NC_CACHE
    if _NC_CACHE is None:
        _NC_CACHE = _build_program()
    return _NC_CACHE


def make_in_maps(inputs):
    h = np.ascontiguousarray(inputs["h"], dtype=np.float32)
    ws = np.asarray(inputs["w_source"], dtype=np.float32)
    wt = np.asarray(inputs["w_target"], dtype=np.float32)
    conv_w = np.asarray(inputs["conv_w"], dtype=np.float32)
    conv_b = np.asarray(inputs["conv_b"], dtype=np.float32)
    fc_k_w = np.asarray(inputs["fc_k_w"], dtype=np.float32)
    fc_k_b = np.asarray(inputs["fc_k_b"], dtype=np.float32)
    fc_w = np.asarray(inputs["fc_w"], dtype=np.float32)
    fc_b = np.asarray(inputs["fc_b"], dtype=np.float32)
    in_w = np.asarray(inputs["in_w"], dtype=np.float32)
    in_b = np.asarray(inputs["in_b"], dtype=np.float32)

    h_bf = np.ascontiguousarray(h).astype(BF)
    ws_t = np.ascontiguousarray(ws.transpose(0, 2, 1))
    wt_t = np.ascontiguousarray(wt.transpose(0, 2, 1))
    ws_t_bf = ws_t.astype(BF)
    wt_c = np.ascontiguousarray(wt)

    Wc = np.ascontiguousarray(conv_w[:, :, 0, 0])           # [Q, C]
    M1 = fc_k_w.T @ Wc                                      # [D, C]
    v1 = fc_k_b @ Wc                                        # [C]
    u = fc_k_w.T @ conv_b                                   # [D]
    w0 = float(fc_k_b @ conv_b)

    shared = {
        "m1_bf": np.ascontiguousarray(M1).astype(BF),
        "v1_4": np.ascontiguousarray(v1.reshape(4, 128).T),
        "u4": np.ascontiguousarray(u.reshape(4, 128).T),
        "w0_col": np.full((128, 1), w0, dtype=np.float32),
        "fw_bf": np.ascontiguousarray(fc_w.T).astype(BF),
        "fcb_row_bf": fc_b.reshape(1, 2 * C).astype(BF),
        "ones1_bf": np.ones((1, L), dtype=BF),
        "inw_col": np.ascontiguousarray(in_w.reshape(4, 128).T),
        "inb_col": np.ascontiguousarray(in_b.reshape(4, 128).T),
        "eps_zero": np.tile(np.array([EPS, 0.0], dtype=np.float32), (128, 1)),
        "identity64": np.eye(L, dtype=np.float32),
        "negid64": -np.eye(L, dtype=np.float32),
        "ones64": np.ones((L, L), dtype=np.float32),
        "identity128f": np.eye(128, dtype=np.float32),
        "identity128b": np.eye(128, dtype=BF),
        "sel8_bf": np.repeat(np.eye(8, dtype=BF), L, axis=1),
    }
    in_maps = []
    for i in range(N_CORES):
        lo = i * S
        in_maps.append({
            "h_bf": h_bf[lo:lo + S],
            "ws_t": ws_t[lo:lo + S],
            "wt_t": wt_t[lo:lo + S],
            "ws_t_bf": ws_t_bf[lo:lo + S],
            "wt": wt_c[lo:lo + S],
            **shared,
        })
    return in_maps


def kernel(**inputs):
    in_maps = make_in_maps(inputs)
    nc = _get_nc()
    res = run_bass_kernel_spmd(nc, in_maps, core_ids=list(range(N_CORES)))
    out = np.concatenate([res.results[i]["out"] for i in range(N_CORES)],
                         axis=0)
    return out.astype(np.float32)


if __name__ == "__main__":
    rng = np.random.default_rng(0)
    ins = {
        "h": rng.standard_normal((B, C, H, W), dtype=np.float32),
        "w_source": rng.standard_normal((B, L, D), dtype=np.float32),
        "w_target": rng.standard_normal((B, L, D), dtype=np.float32),
        "conv_w": (rng.standard_normal((Q, C, 1, 1), dtype=np.float32)
                   / np.sqrt(C)),
        "conv_b": np.zeros(Q, np.float32),
        "fc_k_w": (rng.standard_normal((Q, D), dtype=np.float32)
                   / np.sqrt(D)),
        "fc_k_b": np.zeros(Q, np.float32),
        "fc_w": (rng.standard_normal((2 * C, D), dtype=np.float32)
                 / np.sqrt(D)),
        "fc_b": np.zeros(2 * C, np.float32),
        "in_w": np.ones(C, np.float32),
        "in_b": np.zeros(C, np.float32),
    }
    out = kernel(**ins)
    print("out", out.shape, out.dtype, float(np.abs(out).max()))


# revision 24
# speedup vs baseline: 1.0350x; 1.0350x over previous
"""Trainium2 Bass kernel for nn_MACAM (cross-attn modulation + instance norm).

Pure data parallel: batch B=16, 2 samples per core over 8 NeuronCores.
bf16 everywhere except the argmax/scores block (kept fp32 for exact argmax).

  - h shipped bf16 (halves input DMA); output written bf16, host converts back
    to fp32; weights folded+cast on host: M1 = fc_k_w.T @ conv_w so
    kc = ws@M1 + v1, attn = kcT-chunks @ h + kb, kb = ws@u + w0.
  - instance norm folded into the modulation weights:
        gamma2[l,c] = gamma[l,c]*alpha[c]
        beta2[l,c]  = beta[l,c] + gamma[l,c]*delta[c]
        out = h (.) (gamma2.T@attn) + beta2.T@attn
  - argmax/gather as a one-hot matmul (is_equal against the row max).
  - PSUM lives in two 4-bank FIFO pools ([128,1024] f32 slots) shared by all
    phases; modulation runs 1024-wide pairs with a LAG-2 software pipeline so
    the PE never waits on the DVE multiply.
  - PSUM egress on ACT; the final add is split across Pool/PE(id-matmul)/DVE
    to balance engines; sample-1 front work (scores/attn/bn_stats) is emitted
    into hook slots inside sample-0's modulation loop to keep PE/DVE packed.
  - NOTE: sustained 8-core load trips the activity throttler (PE clamped to
    ~1.2 GHz after ~10us); per-engine budgets assume the clamped clock.
    fp8 maps were tested offline and fail the 2e-2 gate (~4e-2); bf16 path
    measures ~6.7e-3.
"""

import os
import sys

os.environ.setdefault("MYCRO_LOCAL_CACHE", "1")
sys.path.insert(0, "/opt/trn_rl_repo")

import ml_dtypes
import numpy as np

import concourse.bacc as bacc
import concourse.bass as bass
import concourse.mybir as mybir
import concourse.tile as tile
from concourse.bass_utils import run_bass_kernel_spmd

N_CORES = 8
B, C, H, W = 16, 512, 64, 64
HW = H * W
L, D, Q = 64, 512, 512
S = B // N_CORES          # samples per core
EPS = 1e-5
NP = 8                    # HW pieces of 512
NC4 = 4                   # channel chunks of 128

f32 = mybir.dt.float32
bf16 = mybir.dt.bfloat16
AF = mybir.ActivationFunctionType
ALU = mybir.AluOpType
AX = mybir.AxisListType
BF = ml_dtypes.bfloat16


def _build_program():
    nc = bacc.Bacc("TRN2", target_bir_lowering=False, debug=False,
                   num_devices=N_CORES)
    dt_ = nc.dram_tensor
    h_d = dt_("h_bf", [S, C, H, W], bf16, kind="ExternalInput").ap()
    wsT_d = dt_("ws_t", [S, D, L], f32, kind="ExternalInput").ap()
    wtT_d = dt_("wt_t", [S, D, L], f32, kind="ExternalInput").ap()
    wsTb_d = dt_("ws_t_bf", [S, D, L], bf16, kind="ExternalInput").ap()
    wt_d = dt_("wt", [S, L, D], f32, kind="ExternalInput").ap()
    m1_d = dt_("m1_bf", [D, C], bf16, kind="ExternalInput").ap()
    v14_d = dt_("v1_4", [128, 4], f32, kind="ExternalInput").ap()
    u4_d = dt_("u4", [128, 4], f32, kind="ExternalInput").ap()
    w0_d = dt_("w0_col", [128, 1], f32, kind="ExternalInput").ap()
    fw_d = dt_("fw_bf", [D, 2 * C], bf16, kind="ExternalInput").ap()
    fcb_d = dt_("fcb_row_bf", [1, 2 * C], bf16, kind="ExternalInput").ap()
    ones1_d = dt_("ones1_bf", [1, L], bf16, kind="ExternalInput").ap()
    inw_d = dt_("inw_col", [128, 4], f32, kind="ExternalInput").ap()
    inb_d = dt_("inb_col", [128, 4], f32, kind="ExternalInput").ap()
    epz_d = dt_("eps_zero", [128, 2], f32, kind="ExternalInput").ap()
    id64_d = dt_("identity64", [L, L], f32, kind="ExternalInput").ap()
    nid64_d = dt_("negid64", [L, L], f32, kind="ExternalInput").ap()
    ones64_d = dt_("ones64", [L, L], f32, kind="ExternalInput").ap()
    id128f_d = dt_("identity128f", [128, 128], f32, kind="ExternalInput").ap()
    id128b_d = dt_("identity128b", [128, 128], bf16, kind="ExternalInput").ap()
    sel8_d = dt_("sel8_bf", [8, 8 * L], bf16, kind="ExternalInput").ap()
    out_d = dt_("out", [S, C, H, W], bf16, kind="ExternalOutput").ap()

    h_v = h_d.rearrange("s (n p) a b -> s n p (a b)", p=128)     # [S,4,128,4096]
    out_v = out_d.rearrange("s (n p) a b -> s n p (a b)", p=128)
    wsT_v = wsT_d.rearrange("s (n p) l -> s p n l", p=128)       # [S,128,4,64]
    wtT_v = wtT_d.rearrange("s (n p) l -> s p n l", p=128)
    wsTb_v = wsTb_d.rearrange("s (n p) l -> s p n l", p=128)

    with tile.TileContext(nc) as tc:
        with (
            tc.tile_pool(name="wpool", bufs=1) as wpool,
            tc.tile_pool(name="hpool", bufs=8) as hpool,
            tc.tile_pool(name="attnpool", bufs=2) as attnpool,
            tc.tile_pool(name="spool", bufs=2) as spool,
            tc.tile_pool(name="piece", bufs=3) as piece,
            tc.tile_pool(name="stage", bufs=2) as stage,
            tc.tile_pool(name="ps_g", bufs=2, space="PSUM") as ps_g,
            tc.tile_pool(name="ps_b", bufs=2, space="PSUM") as ps_b,
        ):
            def gslot():
                t = ps_g.tile([128, 1024], f32, tag="g", name="gs")
                return t

            def bslot():
                t = ps_b.tile([128, 1024], f32, tag="b", name="bs")
                return t

            # ---- persistent weights ----
            m1_t = []     # M1 chunks [128(d), 512(c)] bf16
            fw_t = []     # fc_w.T chunks [128(d), 1024] bf16
            for j in range(4):
                t = wpool.tile([128, C], bf16, tag=f"m1{j}", name=f"m1{j}")
                nc.scalar.dma_start(t[:], m1_d[j * 128:(j + 1) * 128, :])
                m1_t.append(t)
            for j in range(4):
                t = wpool.tile([128, 2 * C], bf16, tag=f"fw{j}", name=f"fw{j}")
                nc.scalar.dma_start(t[:], fw_d[j * 128:(j + 1) * 128, :])
                fw_t.append(t)
            v1_col = wpool.tile([128, 4], f32, tag="v1")
            nc.gpsimd.dma_start(v1_col[:], v14_d)
            u_col = wpool.tile([128, 4], f32, tag="u")
            nc.gpsimd.dma_start(u_col[:], u4_d)
            w0_col = wpool.tile([128, 1], f32, tag="w0")
            nc.gpsimd.dma_start(w0_col[:], w0_d)
            fcb_row = wpool.tile([1, 2 * C], bf16, tag="fcb")
            nc.gpsimd.dma_start(fcb_row[:], fcb_d)
            ones1b = wpool.tile([1, L], bf16, tag="ones1b")
            nc.gpsimd.dma_start(ones1b[:], ones1_d)
            inw_col = wpool.tile([128, 4], f32, tag="inw")
            nc.gpsimd.dma_start(inw_col[:], inw_d)
            inb_col = wpool.tile([128, 4], f32, tag="inb")
            nc.gpsimd.dma_start(inb_col[:], inb_d)
            epz = wpool.tile([128, 2], f32, tag="epz")
            nc.gpsimd.dma_start(epz[:], epz_d)
            id64 = wpool.tile([L, L], f32, tag="id64")
            nc.gpsimd.dma_start(id64[:], id64_d)
            nid64 = wpool.tile([L, L], f32, tag="nid64")
            nc.gpsimd.dma_start(nid64[:], nid64_d)
            ones64 = wpool.tile([L, L], f32, tag="ones64")
            nc.gpsimd.dma_start(ones64[:], ones64_d)
            id128f = wpool.tile([128, 128], f32, tag="id128f")
            nc.gpsimd.dma_start(id128f[:], id128f_d)
            id128b = wpool.tile([128, 128], bf16, tag="id128b")
            nc.gpsimd.dma_start(id128b[:], id128b_d)
            sel8 = wpool.tile([8, 8 * L], bf16, tag="sel8")
            nc.gpsimd.dma_start(sel8[:], sel8_d)

            def load_h(s):
                h_t = []
                for cc in range(NC4):
                    t = hpool.tile([128, HW], bf16, tag="h", name="h")
                    nc.sync.dma_start(t[:], h_v[s, cc])
                    h_t.append(t)
                return h_t

            def fA(s, h_t=None):
                st = {}
                st["h_t"] = h_t if h_t is not None else load_h(s)

                # ---- per-sample small inputs ----
                wsT4 = spool.tile([128, 4 * L], f32, tag="wsT4")
                nc.gpsimd.dma_start(
                    wsT4[:].rearrange("p (n l) -> p n l", l=L), wsT_v[s])
                wtT4 = spool.tile([128, 4 * L], f32, tag="wtT4")
                nc.gpsimd.dma_start(
                    wtT4[:].rearrange("p (n l) -> p n l", l=L), wtT_v[s])
                wsT4b = spool.tile([128, 4 * L], bf16, tag="wsT4b")
                nc.gpsimd.dma_start(
                    wsT4b[:].rearrange("p (n l) -> p n l", l=L), wsTb_v[s])
                wt_sb = spool.tile([L, D], f32, tag="wt_sb")
                nc.gpsimd.dma_start(wt_sb[:], wt_d[s])
                st["wsT4"] = wsT4
                st["st_col"] = spool.tile([128, 8], f32, tag="st_col",
                                          name="st_col")

                # ---- masking allocation (fp32): scores -> one-hot P ----
                scores_ps = gslot()[0:L, 0:L]
                for j in range(4):
                    nc.tensor.matmul(
                        scores_ps,
                        wsT4[:, j * L:(j + 1) * L], wtT4[:, j * L:(j + 1) * L],
                        start=(j == 0), stop=(j == 3))
                scores_sb = spool.tile([L, L], f32, tag="scores_sb")
                nc.scalar.copy(scores_sb[:], scores_ps)
                colsum_ps = bslot()[0:1, 0:L]
                nc.tensor.matmul(colsum_ps, ones64[:, 0:1], scores_sb[:],
                                 start=True, stop=True)
                colsum_row = spool.tile([1, L], f32, tag="colsum")
                nc.scalar.copy(colsum_row[:], colsum_ps)
                left_ps = gslot()[0:L, 0:L]
                nc.tensor.matmul(left_ps, ones64[0:1, :], colsum_row[:],
                                 start=True, stop=False)
                nc.tensor.matmul(left_ps, nid64[:], scores_sb[:],
                                 start=False, stop=True)
                rowmax = spool.tile([L, 1], f32, tag="rowmax")
                nc.vector.tensor_reduce(rowmax[:], left_ps, AX.X, ALU.max)
                P_sb = spool.tile([L, L], f32, tag="P_sb")
                nc.vector.tensor_scalar(P_sb[:], left_ps, rowmax[:], None,
                                        ALU.is_equal)
                PT_ps = bslot()[0:L, 0:L]
                nc.tensor.transpose(PT_ps, P_sb[:], id64[:])
                PT_sb = spool.tile([L, L], f32, tag="PT_sb")
                nc.scalar.copy(PT_sb[:], PT_ps)

                # ---- w_allocT = wt.T @ P.T, cast to bf16 ----
                waT_pack = bslot()
                waT_bf = spool.tile([128, 4 * L], bf16, tag="waT_bf")
                for j in range(4):
                    nc.tensor.matmul(waT_pack[:, j * L:(j + 1) * L],
                                     wt_sb[:, j * 128:(j + 1) * 128],
                                     PT_sb[:], start=True, stop=True)
                nc.scalar.copy(waT_bf[:], waT_pack[:, 0:4 * L])

                # ---- bg = w_alloc @ fc_w.T + fc_b ----
                beta_sb = spool.tile([L, C], f32, tag="beta_sb")
                gamma_sb = spool.tile([L, C], f32, tag="gamma_sb")
                for half, dst in ((0, beta_sb), (1, gamma_sb)):
                    bg_ps = bslot()[0:L, 0:C]
                    for j in range(4):
                        nc.tensor.matmul(
                            bg_ps, waT_bf[:, j * L:(j + 1) * L],
                            fw_t[j][:, half * C:(half + 1) * C],
                            start=(j == 0), stop=False)
                    nc.tensor.matmul(bg_ps, ones1b[:],
                                     fcb_row[:, half * C:(half + 1) * C],
                                     start=False, stop=True)
                    nc.scalar.copy(dst[:], bg_ps)
                st["beta_sb"] = beta_sb
                st["gamma_sb"] = gamma_sb

                # ---- kcT = M1.T @ ws.T + v1 ----
                kcT_pack = bslot()
                kcT_bf = spool.tile([128, 4 * L], bf16, tag="kcT_bf")
                for cc in range(NC4):
                    for j in range(4):
                        nc.tensor.matmul(
                            kcT_pack[:, cc * L:(cc + 1) * L],
                            m1_t[j][:, cc * 128:(cc + 1) * 128],
                            wsT4b[:, j * L:(j + 1) * L],
                            start=(j == 0), stop=(j == 3))
                    nc.scalar.activation(kcT_bf[:, cc * L:(cc + 1) * L],
                                         kcT_pack[:, cc * L:(cc + 1) * L],
                                         AF.Identity,
                                         bias=v1_col[:, cc:cc + 1])
                st["kcT_bf"] = kcT_bf

                # ---- kb = ws @ u + w0 ----
                kb_ps = bslot()[0:L, 0:1]
                for j in range(4):
                    nc.tensor.matmul(kb_ps, wsT4[:, j * L:(j + 1) * L],
                                     u_col[:, j:j + 1],
                                     start=(j == 0), stop=(j == 3))
                kb_col = spool.tile([L, 1], f32, tag="kb_col")
                nc.scalar.activation(kb_col[:], kb_ps, AF.Identity,
                                     bias=w0_col[0:L, :])
                st["kb_col"] = kb_col
                st["attn_bf"] = attnpool.tile([L, HW], bf16, tag="attn_bf",
                                              name="attn_bf")
                return st

            def attn_piece(st, p2):
                attn2 = gslot()
                for half in range(2):
                    pp = 2 * p2 + half
                    for cc in range(NC4):
                        nc.tensor.matmul(
                            attn2[0:L, half * 512:(half + 1) * 512],
                            st["kcT_bf"][:, cc * L:(cc + 1) * L],
                            st["h_t"][cc][:, pp * 512:(pp + 1) * 512],
                            start=(cc == 0), stop=(cc == 3))
                nc.scalar.activation(
                    st["attn_bf"][:, p2 * 1024:(p2 + 1) * 1024],
                    attn2[0:L, :], AF.Identity, bias=st["kb_col"][:])

            def stats_chunk(st, cc):
                st_col = st["st_col"]
                st6 = spool.tile([128, 48], f32, tag="st6", name="st6")
                for k in range(8):
                    nc.vector.bn_stats(
                        st6[:, k * 6:(k + 1) * 6],
                        st["h_t"][cc][:, k * 512:(k + 1) * 512])
                mv = spool.tile([128, 2], f32, tag="mv", name="mv")
                nc.vector.bn_aggr(mv[:], st6[:])
                sd = spool.tile([128, 1], f32, tag="sd", name="sd")
                nc.scalar.activation(sd[:], mv[:, 1:2], AF.Sqrt,
                                     bias=epz[:, 0:1])
                rs = spool.tile([128, 1], f32, tag="rs", name="rs")
                nc.vector.reciprocal(rs[:], sd[:])
                nc.vector.tensor_tensor(
                    st_col[:, cc:cc + 1], rs[:], inw_col[:, cc:cc + 1],
                    ALU.mult)
                ms = spool.tile([128, 1], f32, tag="ms", name="ms")
                nc.vector.tensor_tensor(ms[:], mv[:, 0:1],
                                        st_col[:, cc:cc + 1], ALU.mult)
                nc.vector.tensor_tensor(st_col[:, 4 + cc:5 + cc],
                                        inb_col[:, cc:cc + 1], ms[:],
                                        ALU.subtract)

            def folds(st):
                stT_ps = bslot()[0:8, 0:128]
                nc.tensor.transpose(stT_ps, st["st_col"][:], id128f[:])
                st8b = spool.tile([8, 128], bf16, tag="st8b")
                nc.scalar.copy(st8b[:], stT_ps)
                sm_ps = gslot()[0:L, 0:C]
                t2m_ps = bslot()[0:L, 0:C]
                for j in range(4):
                    nc.tensor.matmul(sm_ps[:, j * 128:(j + 1) * 128],
                                     sel8[:, j * L:(j + 1) * L], st8b[:],
                                     start=True, stop=True)
                    nc.tensor.matmul(t2m_ps[:, j * 128:(j + 1) * 128],
                                     sel8[:, (4 + j) * L:(5 + j) * L], st8b[:],
                                     start=True, stop=True)
                gamma2_bf = spool.tile([L, C], bf16, tag="gamma2_bf",
                                       name="gamma2_bf")
                nc.vector.tensor_tensor(gamma2_bf[:], st["gamma_sb"][:],
                                        sm_ps[:], ALU.mult)
                tg_sb = spool.tile([L, C], f32, tag="tg_sb")
                nc.vector.tensor_tensor(tg_sb[:], st["gamma_sb"][:],
                                        t2m_ps[:], ALU.mult)
                beta2_bf = spool.tile([L, C], bf16, tag="beta2_bf",
                                      name="beta2_bf")
                nc.vector.tensor_tensor(beta2_bf[:], st["beta_sb"][:],
                                        tg_sb[:], ALU.add)
                st["gamma2_bf"] = gamma2_bf
                st["beta2_bf"] = beta2_bf

            def maps(s, st, hooks=None):
                h_t = st["h_t"]
                attn_bf = st["attn_bf"]
                gamma2_bf = st["gamma2_bf"]
                beta2_bf = st["beta2_bf"]
                # ---- modulation: 1024-wide pairs, LAG-deep software pipeline
                # (PE never waits on the DVE mult; PSUM slots recycle 2 pairs
                # = 4 pieces apart) ----
                LAG = 2
                pairs = [(cc, p2) for cc in range(NC4) for p2 in range(4)]
                stage_t = {}
                tmp_tiles = {}
                for t in range(len(pairs) + LAG):
                    if hooks and t in hooks:
                        for fn in hooks.pop(t):
                            fn()
                    if t < len(pairs):
                        cc, p2 = pairs[t]
                        if p2 == 0:
                            stage_t[cc] = stage.tile([128, HW], bf16,
                                                     tag="stage", name="stg")
                        gm2 = gslot()
                        for half in range(2):
                            pp = 2 * p2 + half
                            nc.tensor.matmul(
                                gm2[:, half * 512:(half + 1) * 512],
                                gamma2_bf[:, cc * 128:(cc + 1) * 128],
                                attn_bf[:, pp * 512:(pp + 1) * 512],
                                start=True, stop=True)
                        tmp2 = piece.tile([128, 1024], bf16, tag="tmp")
                        nc.vector.tensor_tensor(
                            tmp2[:], h_t[cc][:, p2 * 1024:(p2 + 1) * 1024],
                            gm2[:], ALU.mult)
                        tmp_tiles[t] = tmp2
                    if t >= LAG:
                        i = t - LAG
                        cc, p2 = pairs[i]
                        # adds: mostly Pool; 2/sample on PE (id-matmul),
                        # 2/sample on DVE (bf16 2x) to balance engines
                        if hooks is not None:
                            # stats of the next sample share the DVE here
                            eng = "pe" if i % 4 == 3 else "pool"
                        else:
                            eng = "pe" if i % 8 == 3 else (
                                "dve" if (i % 4 == 3 or i >= 13) else "pool")
                        t2 = tmp_tiles.pop(i)
                        sl = stage_t[cc][:, p2 * 1024:(p2 + 1) * 1024]
                        bm2 = bslot()
                        for half in range(2):
                            pp = 2 * p2 + half
                            nc.tensor.matmul(
                                bm2[:, half * 512:(half + 1) * 512],
                                beta2_bf[:, cc * 128:(cc + 1) * 128],
                                attn_bf[:, pp * 512:(pp + 1) * 512],
                                start=True, stop=(eng != "pe"))
                        if eng == "pe":
                            for half in range(2):
                                nc.tensor.matmul(
                                    bm2[:, half * 512:(half + 1) * 512],
                                    id128b[:],
                                    t2[:, half * 512:(half + 1) * 512],
                                    start=False, stop=True)
                            nc.scalar.copy(sl, bm2[:])
                        else:
                            bmb = piece.tile([128, 1024], bf16, tag="bmb")
                            nc.scalar.copy(bmb[:], bm2[:])
                            e = nc.gpsimd if eng == "pool" else nc.vector
                            e.tensor_tensor(sl, bmb[:], t2[:], ALU.add)
                        nc.sync.dma_start(
                            out_v[s, cc][:, p2 * 1024:(p2 + 1) * 1024],
                            stage_t[cc][:, p2 * 1024:(p2 + 1) * 1024])

            st0 = fA(0)
            h1 = load_h(1)
            for p2 in range(4):
                attn_piece(st0, p2)
            stats_chunk(st0, 0)
            for cc in range(1, NC4):
                stats_chunk(st0, cc)
            folds(st0)

            st1 = {}

            def _fa1():
                st1.update(fA(1, h1))

            hooks = {
                1: [_fa1],
                2: [lambda: stats_chunk(st1, 0)],
                4: [lambda: attn_piece(st1, 0)],
                5: [lambda: stats_chunk(st1, 1)],
                7: [lambda: attn_piece(st1, 1)],
                8: [lambda: stats_chunk(st1, 2)],
                10: [lambda: attn_piece(st1, 2)],
                11: [lambda: stats_chunk(st1, 3)],
                13: [lambda: attn_piece(st1, 3)],
                15: [lambda: folds(st1)],
            }
            maps(0, st0, hooks)
            maps(1, st1)

    nc.compile()
    return nc


_NC_CACHE = None


def _get_nc():
    global _NC_CACHE
    if _NC_CACHE is None:
        _NC_CACHE = _build_program()
    return _NC_CACHE


def make_in_maps(inputs):
    h = np.ascontiguousarray(inputs["h"], dtype=np.float32)
    ws = np.asarray(inputs["w_source"], dtype=np.float32)
    wt = np.asarray(inputs["w_target"], dtype=np.float32)
    conv_w = np.asarray(inputs["conv_w"], dtype=np.float32)
    conv_b = np.asarray(inputs["conv_b"], dtype=np.float32)
    fc_k_w = np.asarray(inputs["fc_k_w"], dtype=np.float32)
    fc_k_b = np.asarray(inputs["fc_k_b"], dtype=np.float32)
    fc_w = np.asarray(inputs["fc_w"], dtype=np.float32)
    fc_b = np.asarray(inputs["fc_b"], dtype=np.float32)
    in_w = np.asarray(inputs["in_w"], dtype=np.float32)
    in_b = np.asarray(inputs["in_b"], dtype=np.float32)

    h_bf = np.ascontiguousarray(h).astype(BF)
    ws_t = np.ascontiguousarray(ws.transpose(0, 2, 1))
    wt_t = np.ascontiguousarray(wt.transpose(0, 2, 1))
    ws_t_bf = ws_t.astype(BF)
    wt_c = np.ascontiguousarray(wt)

    Wc = np.ascontiguousarray(conv_w[:, :, 0, 0])           # [Q, C]
    M1 = fc_k_w.T @ Wc                                      # [D, C]
    v1 = fc_k_b @ Wc                                        # [C]
    u = fc_k_w.T @ conv_b                                   # [D]
    w0 = float(fc_k_b @ conv_b)

    shared = {
        "m1_bf": np.ascontiguousarray(M1).astype(BF),
        "v1_4": np.ascontiguousarray(v1.reshape(4, 128).T),
        "u4": np.ascontiguousarray(u.reshape(4, 128).T),
        "w0_col": np.full((128, 1), w0, dtype=np.float32),
        "fw_bf": np.ascontiguousarray(fc_w.T).astype(BF),
        "fcb_row_bf": fc_b.reshape(1, 2 * C).astype(BF),
        "ones1_bf": np.ones((1, L), dtype=BF),
        "inw_col": np.ascontiguousarray(in_w.reshape(4, 128).T),
        "inb_col": np.ascontiguousarray(in_b.reshape(4, 128).T),
        "eps_zero": np.tile(np.array([EPS, 0.0], dtype=np.float32), (128, 1)),
        "identity64": np.eye(L, dtype=np.float32),
        "negid64": -np.eye(L, dtype=np.float32),
        "ones64": np.ones((L, L), dtype=np.float32),
        "identity128f": np.eye(128, dtype=np.float32),
        "identity128b": np.eye(128, dtype=BF),
        "sel8_bf": np.repeat(np.eye(8, dtype=BF), L, axis=1),
    }
    in_maps = []
    for i in range(N_CORES):
        lo = i * S
        in_maps.append({
            "h_bf": h_bf[lo:lo + S],
            "ws_t": ws_t[lo:lo + S],
            "wt_t": wt_t[lo:lo + S],
            "ws_t_bf": ws_t_bf[lo:lo + S],
            "wt": wt_c[lo:lo + S],
            **shared,
        })
    return in_maps


def kernel(**inputs):
    in_maps = make_in_maps(inputs)
    nc = _get_nc()
    res = run_bass_kernel_spmd(nc, in_maps, core_ids=list(range(N_CORES)))
    out = np.concatenate([res.results[i]["out"] for i in range(N_CORES)],
                         axis=0)
    return out.astype(np.float32)


if __name__ == "__main__":
    rng = np.random.default_rng(0)
    ins = {
        "h": rng.standard_normal((B, C, H, W), dtype=np.float32),
        "w_source": rng.standard_normal((B, L, D), dtype=np.float32),
        "w_target": rng.standard_normal((B, L, D), dtype=np.float32),
        "conv_w": (rng.standard_normal((Q, C, 1, 1), dtype=np.float32)
                   / np.sqrt(C)),
        "conv_b": np.zeros(Q, np.float32),
        "fc_k_w": (rng.standard_normal((Q, D), dtype=np.float32)
                   / np.sqrt(D)),
        "fc_k_b": np.zeros(Q, np.float32),
        "fc_w": (rng.standard_normal((2 * C, D), dtype=np.float32)
                 / np.sqrt(D)),
        "fc_b": np.zeros(2 * C, np.float32),
        "in_w": np.ones(C, np.float32),
        "in_b": np.zeros(C, np.float32),
    }
    out = kernel(**ins)
    print("out", out.shape, out.dtype, float(np.abs(out).max()))
